# revision 25
# baseline (speedup 1.0000x reference)
"""Bass/Trainium2 kernel for nn_BlastocystAuxLoss.

Computes a masked MSE over B=16,777,216 elements:
    late stages are labels 8..15; target[s] = (s-8) * 4/7 for late stages;
    loss = sum_{s>=8} (x - target)^2 / count(s>=8)   (0.0 if count == 0)

Strategy: trivially data-parallel over 8 NeuronCores; each core streams its
B/8 shard from HBM. Host-side the inputs are re-typed to cut HBM traffic
from 8 B/elem to 3 B/elem: scores fp32 -> fp16 (well within the 2e-2
tolerance), labels int32 -> int8 (values 0..15, lossless).

Per-element pipeline (s = label, x = score):
    ACT : w = Prelu(4/7*s - 32/7, alpha=256)    # = target for s>=8,
                                                # <= -146 for s<8
    DVE : q = min((x - w)^2, 4096), accum -> sum(q)   [one fused custom op]
          early elements have x-w >= +140, so q == 4096.0 exactly
    e   : early indicator (q >= 2048), summed per tile on whichever engine
          has slack: TensorE ones-matmul (big early tiles), exact
          sigmoid+accum on ACT (the two largest tiles), is_ge+accum on DVE
          (the small tail tiles, avoiding the tail matmul->PSUM chain)

    sse  = sum(q) - 4096 * n_early              # host, f64
    cnt  = B - n_early
The fused DVE op (sub+square+clamp+reduce in one pass) is registered into
concourse's custom-DVE table at import time (additive registration through
the framework's own extension mechanism; rows 17+ of the 5-bit opcode
space are unused).
"""

from contextlib import ExitStack
from operator import add as _op_add

import numpy as np

B = 16777216
N_CORES = 8
SHARD = B // N_CORES  # 2,097,152
P = 128

ALPHA = 256.0  # Prelu negative slope: early w <= -alpha*4/7 ~ -146
QCLAMP = 4096.0  # clamp for (x-w)^2: early hits it exactly
ETHRESH = 2048.0  # q >= 2048 <=> early (late q <= ~120)
MMCHUNK = 512  # matmul free-dim chunk (one PSUM bank)

SIZES = [512, 1536, 2560, 4096, 4096, 2048, 1024, 512]
E_ACT = (3, 4)  # e via ACT sigmoid+accum (the 4096 tiles)
E_DVE = (6, 7)  # e via DVE is_ge+accum (tail tiles; no matmul dependency)
E_MM = (0, 1, 2, 5)  # e via TensorE ones-matmul into PSUM

_NC_CACHE = {}


def _register_custom_op():
    """Register the fused  out = min((in0-in1)^2, c0), accum = sum(out)
    DVE op into concourse.dve_ops (idempotent, additive)."""
    from concourse import dve_ops
    from concourse.dve_spec import C0, Spec, Src0, Src1, lower, minn, sq
    from concourse.dve_uop import DveOpSpec

    name = "CLAMP_SQ_DIFF_REDUCE_ANT"
    for op in dve_ops.OPS:
        if op.name == name:
            return op

    def _ref(in0, in1, s0, s1, imm2):
        b = np.minimum(
            (in0.astype(np.float32) - in1.astype(np.float32)) ** 2,
            np.float32(s0),
        ).astype(np.float32)
        return b, b.reshape(b.shape[0], -1).sum(axis=-1, keepdims=True)

    spec = Spec(body=minn(sq(Src0 - Src1), C0), accum=_op_add, reference=_ref)
    shas = {}
    for ver in ("v3", "v4"):
        s = DveOpSpec(name=name, opcode=0, uops=lower(spec, ver=ver), rd1_en=True)
        shas[ver] = s.sha(ver)
    op = dve_ops.DveOp(name, spec, subdim=False, uops_sha=shas)
    dve_ops.OPS.append(op)
    dve_ops.CUSTOM_DVE_SPECS[name] = spec
    dve_ops._SUB_OPCODE_FOR_NAME[name] = (
        max(dve_ops._SUB_OPCODE_FOR_NAME.values()) + 1
    )
    assert dve_ops._SUB_OPCODE_FOR_NAME[name] < 0x20
    return op


def build_raw(shard=SHARD):
    """Hand-scheduled raw-Bass builder (no TileContext). All DMAs are
    issued up-front (full ring, no slot reuse)."""
    import concourse.bacc as bacc
    from concourse import mybir

    fused_op = _register_custom_op()

    free = shard // P
    sizes = list(SIZES)
    e_act, e_dve, e_mm = set(E_ACT), set(E_DVE), set(E_MM)
    if sum(sizes) != free:  # non-default shard (tests)
        fd = free // 8
        sizes = [fd] * 8
    assert sum(sizes) == free
    fd = max(sizes)
    NT = len(sizes)
    offs = [sum(sizes[:i]) for i in range(NT)]
    NEXTRA = len(e_act) + len(e_dve)  # extra acc columns for non-matmul e

    nc = bacc.Bacc("TRN2", target_bir_lowering=False)
    x_ext = nc.declare_dram_parameter(
        "blast_scores", [shard], mybir.dt.float16, isOutput=False
    )
    s_ext = nc.declare_dram_parameter(
        "stage_labels", [shard], mybir.dt.int8, isOutput=False
    )
    out_ext = nc.declare_dram_parameter(
        "out", [P * (NT + NEXTRA)], mybir.dt.float32, isOutput=True
    )
    oute_ext = nc.declare_dram_parameter(
        "out_e", [MMCHUNK], mybir.dt.float32, isOutput=True
    )

    x_v = x_ext.ap().rearrange("(p f) -> p f", p=P)
    s_v = s_ext.ap().rearrange("(p f) -> p f", p=P)

    f32 = mybir.dt.float32
    i8 = mybir.dt.int8
    f16 = mybir.dt.float16
    Alu = mybir.AluOpType
    Act = mybir.ActivationFunctionType

    x_t = [nc.alloc_sbuf_tensor(f"x{i}", [P, w], f16).ap() for i, w in enumerate(sizes)]
    s_t = [nc.alloc_sbuf_tensor(f"s{i}", [P, w], i8).ap() for i, w in enumerate(sizes)]
    w_t = [nc.alloc_sbuf_tensor(f"w{i}", [P, fd], f16).ap() for i in range(5)]
    q_t = [nc.alloc_sbuf_tensor(f"q{i}", [P, fd], f16).ap() for i in range(3)]
    e_t = [nc.alloc_sbuf_tensor(f"e{i}", [P, fd], f16).ap() for i in range(3)]
    ejunk = nc.alloc_sbuf_tensor("ejunk", [P, fd], f16).ap()
    acc = nc.alloc_sbuf_tensor("acc", [P, NT + NEXTRA], f32).ap()
    red = nc.alloc_sbuf_tensor("red", [1, MMCHUNK], f32).ap()
    lr_bias = nc.alloc_sbuf_tensor("lr_bias", [P, 1], f32).ap()
    sg_bias = nc.alloc_sbuf_tensor("sg_bias", [P, 1], f32).ap()
    ones16 = nc.alloc_sbuf_tensor("ones16", [P, 1], f16).ap()
    warm = nc.alloc_sbuf_tensor("warm", [P, 1], f16).ap()

    # engine op orders and cumulative semaphore positions
    # ACT stream: w0..w4, e3, w5, e4, w6, w7, final-copy
    act_order = [("w", 0), ("w", 1), ("w", 2), ("w", 3), ("w", 4)]
    if 3 in e_act:
        act_order.append(("e", 3))
    act_order.append(("w", 5))
    if 4 in e_act:
        act_order.append(("e", 4))
    act_order += [("w", 6), ("w", 7)]
    if NT != 8:  # test fallback: plain order, all e on DVE
        act_order = [("w", k) for k in range(NT)]
        e_act, e_mm = set(), set()
        e_dve = set(range(NT))
    pos_w, pos_esig = {}, {}
    for idx, (kind, k) in enumerate(act_order):
        (pos_w if kind == "w" else pos_esig)[k] = idx + 1

    # DVE stream: memset, then per tile: q(k) [+ e(k) unless e_act]
    pos_q, pos_e = {}, {}
    dvec = 1  # the ones16/bias memsets count once
    for k in range(NT):
        dvec += 1
        pos_q[k] = dvec
        if k not in e_act:
            dvec += 1
            pos_e[k] = dvec
    DVE_TOTAL = dvec

    # e-slot assignment for tiles that materialize e (e_mm + e_dve)
    e_writers = sorted(e_mm | e_dve)
    e_slot = {k: i % 3 for i, k in enumerate(e_writers)}
    mm_index = {k: i + 1 for i, k in enumerate(sorted(e_mm))}  # mm sem after tile k

    # acc column layout: [0..NT) = q sums; then e_act tiles; then e_dve tiles
    ecol = {}
    c = NT
    for k in sorted(e_act):
        ecol[k] = c
        c += 1
    for k in sorted(e_dve):
        ecol[k] = c
        c += 1

    with ExitStack() as ctx:
        dma_x = [ctx.enter_context(nc.semaphore(f"dma_x{i}")) for i in range(NT)]
        dma_s = [ctx.enter_context(nc.semaphore(f"dma_s{i}")) for i in range(NT)]
        dve = ctx.enter_context(nc.semaphore("dve"))
        act = ctx.enter_context(nc.semaphore("act"))
        mm = ctx.enter_context(nc.semaphore("mm"))
        outd = ctx.enter_context(nc.semaphore("outd"))
        rdy = ctx.enter_context(nc.semaphore("rdy"))
        ps_e = ctx.enter_context(nc.psum_tensor("pse", [1, MMCHUNK], f32))
        block = ctx.enter_context(nc.Block())

        @block.sync
        def _(sync):
            for k in range(NT):
                w = sizes[k]
                sync.dma_start(
                    out=s_t[k][:, :w], in_=s_v[:, offs[k] : offs[k] + w]
                ).then_inc(dma_s[k], 16)
                sync.dma_start(
                    out=x_t[k][:, :w], in_=x_v[:, offs[k] : offs[k] + w]
                ).then_inc(dma_x[k], 16)
            # e-psum partials can ship as soon as the copy lands
            sync.wait_ge(act, len(act_order) + 1)
            sync.dma_start(out=oute_ext.ap()[:], in_=red[0:1, :]).then_inc(outd, 16)
            # acc ships once the last DVE op and the ACT e-accums are done
            sync.wait_ge(dve, DVE_TOTAL)
            if e_act:
                sync.wait_ge(act, max(pos_esig[k] for k in e_act))
            sync.dma_start(
                out=out_ext.ap().rearrange("(p f) -> p f", p=P)[:, :], in_=acc[:, :]
            ).then_inc(outd, 16)
            sync.wait_ge(outd, 32)

        @block.scalar
        def _(scalar):
            scalar.wait_ge(rdy, 2)  # both bias tiles ready
            # dummy activation pulls the ACT tables in during the first DMAs
            scalar.activation(
                warm[:, :], lr_bias[:, :], Act.Prelu,
                bias=lr_bias[:, :], scale=1.0, alpha=ALPHA,
            )
            scalar.activation(
                warm[:, :], lr_bias[:, :], Act.Sigmoid, bias=sg_bias[:, :], scale=64.0,
            )
            for kind, k in act_order:
                w = sizes[k]
                if kind == "w":
                    scalar.wait_ge(dma_s[k], 16)
                    if k >= 5:
                        # w slot free when q(k-5) done
                        scalar.wait_ge(dve, pos_q[k - 5])
                    scalar.activation(
                        w_t[k % 5][:, :w], s_t[k][:, :w], Act.Prelu,
                        bias=lr_bias[:, :], scale=4.0 / 7.0, alpha=ALPHA,
                    ).then_inc(act, 1)
                else:
                    # e = sigmoid(64*(q - 2048)) in {0,1}: 1 iff early;
                    # accumulate the tile's early count for free
                    scalar.wait_ge(dve, pos_q[k])
                    scalar.activation(
                        ejunk[:, :w], q_t[k % 3][:, :w], Act.Sigmoid,
                        bias=sg_bias[:, :], scale=64.0,
                        accum_out=acc[:, ecol[k] : ecol[k] + 1],
                    ).then_inc(act, 1)
            # final: PSUM -> SBUF copy once the e-matmuls are done
            scalar.wait_ge(mm, len(mm_index))
            scalar.activation(red[0:1, :], ps_e.ap()[0:1, :], Act.Copy).then_inc(
                act, 1
            )

        @block.vector
        def _(vector):
            vector.memset(lr_bias[:, :], -32.0 / 7.0).then_inc(rdy, 1)
            vector.memset(sg_bias[:, :], -64.0 * ETHRESH).then_inc(rdy, 1)
            vector.memset(ones16[:, :], 1.0).then_inc(dve, 1)
            for k in range(NT):
                w = sizes[k]
                # fused: q = min((x-w)^2, 4096); acc[:, k] = per-partition sum
                vector.wait_ge(dma_x[k], 16)
                vector.wait_ge(act, pos_w[k])
                if k >= 3 and (k - 3) in e_act:
                    # q slot reused from an ACT-e tile: wait for its sigmoid
                    vector.wait_ge(act, pos_esig[k - 3])
                vector._custom_dve(
                    fused_op,
                    out=q_t[k % 3][:, :w],
                    in0=x_t[k][:, :w],
                    in1=w_t[k % 5][:, :w],
                    s0=QCLAMP,
                    accum_out=acc[:, k : k + 1],
                ).then_inc(dve, 1)
                if k in e_act:
                    continue
                # e slot free when its previous matmul consumer is done
                prev = [j for j in e_writers if e_slot[j] == e_slot[k] and j < k]
                if prev and prev[-1] in mm_index:
                    vector.wait_ge(mm, mm_index[prev[-1]])
                if k in e_mm:
                    vector.tensor_scalar(
                        e_t[e_slot[k]][:, :w], q_t[k % 3][:, :w], ETHRESH, 1.0,
                        Alu.is_ge, Alu.mult,
                    ).then_inc(dve, 1)
                else:
                    # tail tiles: e with direct accumulation (1x mode, but
                    # small tiles; avoids the tail matmul+PSUM chain)
                    vector.tensor_scalar(
                        e_t[e_slot[k]][:, :w], q_t[k % 3][:, :w], ETHRESH, 0.0,
                        Alu.is_ge, Alu.add,
                        accum_out=acc[:, ecol[k] : ecol[k] + 1],
                    ).then_inc(dve, 1)

        @block.tensor
        def _(tensor):
            mm_tiles = sorted(e_mm)
            n_chunks_total = sum(
                (sizes[k] + MMCHUNK - 1) // MMCHUNK for k in mm_tiles
            )
            done = 0
            for k in mm_tiles:
                w = sizes[k]
                tensor.wait_ge(dve, pos_e[k])
                c = 0
                while c < w:
                    cw = min(MMCHUNK, w - c)
                    done += 1
                    ins = tensor.matmul(
                        ps_e.ap()[0:1, 0:cw],
                        ones16[:, 0:1],
                        e_t[e_slot[k]][:, c : c + cw],
                        start=(done == 1),
                        stop=(done == n_chunks_total),
                    )
                    c += cw
                ins.then_inc(mm, 1)

    nc.finalize()
    return nc


def run(x, s, **spmd_kwargs):
    """Shard, run on 8 cores, host-reduce. Returns (loss, BassKernelResults)."""
    from concourse.bass_utils import run_bass_kernel_spmd

    if "nc" not in _NC_CACHE:
        _NC_CACHE["nc"] = build_raw()
    nc = _NC_CACHE["nc"]

    x16 = np.ascontiguousarray(x.astype(np.float16))
    s8 = np.ascontiguousarray(s.astype(np.int8))

    in_maps = [
        {
            "blast_scores": x16[i * SHARD : (i + 1) * SHARD],
            "stage_labels": s8[i * SHARD : (i + 1) * SHARD],
        }
        for i in range(N_CORES)
    ]
    res = run_bass_kernel_spmd(nc, in_maps, core_ids=list(range(N_CORES)), **spmd_kwargs)

    NT = len(SIZES)
    n_early = 0.0
    qsum = 0.0
    for r in res.results:
        o = r["out"].astype(np.float64).reshape(P, -1)
        qsum += o[:, :NT].sum()
        n_early += o[:, NT:].sum()
        n_early += r["out_e"].astype(np.float64).sum()
    cnt = float(B) - n_early
    sse = qsum - QCLAMP * n_early
    val = sse / max(cnt, 1.0) if cnt > 0 else 0.0
    return np.asarray(val, dtype=np.float32), res


def kernel(**inputs):
    x = np.ascontiguousarray(np.asarray(inputs["blast_scores"], dtype=np.float32))
    s = np.ascontiguousarray(np.asarray(inputs["stage_labels"], dtype=np.int32))
    assert x.shape == (B,) and s.shape == (B,)
    return run(x, s)[0]
